# revision 53
# baseline (speedup 1.0000x reference)
"""Causal self-attention on 8 trn2 NeuronCores.

Sharding: core c -> (batch b = c // 4, head-group g = c % 4). Each core
computes 4 of the 16 heads for one batch element and the corresponding
slice of the output projection; the host sums the 4 partial projections
per batch and adds the constant bias terms (bv @ Wp.T + bp) exactly.

All matmuls run in bf16 (fp32 PSUM accumulation). The attention@V matmul
is TRANSPOSED relative to the usual formulation: out[q,d] tiles of
[128q, 65] accumulate e_sliceT @ v per (query-tile, key-tile, head), with
a ones-column in v so column 64 collects the softmax denominator on the
same partition as the outputs -- normalization is then a per-partition
reciprocal + tensor_scalar multiply. All 4 heads of a query tile share
one PSUM bank (a single start zero-marks the whole 2KB zero region).

The schedule is software-pipelined at the CHUNK level: scores+exp for
query chunk n+1 (and a few borrowed steps of n+2) are emitted
interleaved into chunk n's attention window so the ACT engine (the only
engine with exp, ~73us of work concentrated in the late causal chunks)
runs a window ahead of the PE's attention matmuls and rarely throttles
them. Causal masking of diagonal key tiles is done by zeroing
sub-diagonal exp(score) entries with affine_select on the idle GPSIMD
engine instead of PE mask matmuls. Output is written bf16 (host sums
partials in f32), halving out-DMA bytes. Startup DMAs are ordered by
first use on explicit queues because the cost model serializes all
transfers on one ~352 B/ns pipe; the end-of-kernel tails borrow the
drained attv PSUM ring and the idle ACT engine for their copies.
"""

import numpy as np
import ml_dtypes

import concourse.bass as bass
import concourse.mybir as mybir
import concourse.tile as tile
from concourse.bass_utils import run_bass_kernel_spmd

B = 2
T = 2048
C = 1024
H = 16
DH = 64
NCORES = 8
GROUPS = 4           # head groups (tensor parallel)
HPG = H // GROUPS    # heads per group = 4
DG = HPG * DH        # head-group width = 256
CHUNK = 512          # query-block size
NCHUNK = T // CHUNK  # 4
F32 = mybir.dt.float32
BF16 = mybir.dt.bfloat16

# schedule tuning knobs
CFG = {
    "fill": 750.0,      # ns of PE filler between score steps
    "borrow_w0": 3,     # S(2) steps pulled into window 0
    "borrow_w1": 5,     # S(3) steps pulled into window 1
    "borrow_late": 2,   # S(3) tail steps pushed into window 3
    "attv_blk": 8,      # key tiles per attv emission unit
    "dma_transp_max_qt": -1,  # DMA XBAR transpose for tails (off: slower)
}


def _patch_tile_drain():
    """This walrus build lowers Drain/NOP to a CTRL with a single sync-wait
    slot; TileContext's kernel-tail drain accumulates one wait per live
    semaphore and fails codegen. Split the waits across single-wait NOPs."""
    import bass_rust
    from concourse.tile import TileContext

    def _drain_and_barrier_split(self, tick_clock, wait_clock):
        probe = self.nc.sync.nop()
        wait_clock.add_sem_waits(
            probe.ins, tile.ScopedClock({None: tick_clock.global_clock})
        )
        waits = list(probe.ins.sync_info.on_wait or [])
        probe.ins.sync_info.on_wait = []
        # distribute the final-value waits across engines; the all-engine
        # barrier below joins them before the semaphore reset
        engines = [self.nc.sync, self.nc.tensor, self.nc.vector,
                   self.nc.scalar, self.nc.gpsimd]
        for i, w in enumerate(waits):
            n = engines[i % len(engines)].nop()
            if n.ins.sync_info is None:
                n.ins.sync_info = bass_rust.SyncInfo(on_wait=[w], on_update=[])
            else:
                n.ins.sync_info.on_wait = [w]
        self.nc.sync.drain()
        self.nc.all_engine_barrier()
        assert self.sems is not None
        popped = self.nc._tile_sem_poison_stack.pop()
        assert popped is self._sem_poison
        self.nc.clear_and_free_semaphores(list(self.sems.allocated().values()))
        self.nc.all_engine_barrier()

    TileContext._drain_and_barrier = _drain_and_barrier_split

    # Same single-wait limit applies to every lowered TPB instruction (the
    # 64B formats carry one EVENTS field). Post-process the BIR JSON before
    # walrus: hoist extra semaphore waits onto same-engine NoOps.
    import json as _json

    import concourse.bass2jax as bass2jax
    import concourse.bass_utils as bass_utils

    if getattr(bass_utils.compile_bir_kernel, "_wait_split", False):
        return

    _orig_compile = bass_utils.compile_bir_kernel

    def _split_multi_waits(bir_json):
        m = _json.loads(bir_json)
        counter = 0
        changed = False
        for fn in m["functions"]:
            for blk in fn["blocks"]:
                new_insts = []
                for inst in blk["instructions"]:
                    si = inst.get("sync_info")
                    waits = (si or {}).get("on_wait") or []
                    sem_waits = [w for w in waits if w.get("sync_type") == "semaphore"]
                    if len(waits) > 1 and len(sem_waits) == len(waits):
                        changed = True
                        for w in waits[:-1]:
                            counter += 1
                            new_insts.append({
                                "name": f"I-wsplit{counter}",
                                "opcode": "NoOp",
                                "engine": inst["engine"],
                                "ins": [],
                                "outs": [],
                                "sync_info": {"on_wait": [w], "on_update": []},
                            })
                        si["on_wait"] = [waits[-1]]
                    new_insts.append(inst)
                blk["instructions"] = new_insts
        if not changed:
            return bir_json
        return _json.dumps(m).encode()

    def _compile_bir_kernel_split(bir_json, tmpdir, neff_name="file.neff"):
        return _orig_compile(_split_multi_waits(bir_json), tmpdir, neff_name=neff_name)

    _compile_bir_kernel_split._wait_split = True
    bass_utils.compile_bir_kernel = _compile_bir_kernel_split
    bass2jax.compile_bir_kernel = _compile_bir_kernel_split


def build_kernel():
    _patch_tile_drain()
    nc = bass.Bass(target_bir_lowering=False, trn_type="TRN2")

    xT = nc.dram_tensor("xT", [C, T], BF16, kind="ExternalInput")
    wq = nc.dram_tensor("wq", [C, DG], BF16, kind="ExternalInput")
    wk = nc.dram_tensor("wk", [C, DG], BF16, kind="ExternalInput")
    wv = nc.dram_tensor("wv", [C, DG], BF16, kind="ExternalInput")
    wp = nc.dram_tensor("wp", [DG, C], BF16, kind="ExternalInput")
    bq = nc.dram_tensor("bq", [DG], F32, kind="ExternalInput")
    bk = nc.dram_tensor("bk", [DG], F32, kind="ExternalInput")
    out = nc.dram_tensor("out", [T, C], BF16, kind="ExternalOutput")

    KO = C // 128            # 8 contraction subtiles for the projections
    MT = DG // 128           # 2 partition tiles for qT/kT and wp rows
    NQT = T // 128           # 16 query/key 128-tiles
    QPC = CHUNK // 128       # 4 query tiles per chunk
    scale = 1.0 / np.sqrt(DH)

    from contextlib import ExitStack

    with tile.TileContext(nc) as tc, ExitStack() as ctx:
        from concourse.masks import make_identity

        const = ctx.enter_context(tc.tile_pool(name="const", bufs=1))
        xt_pool = ctx.enter_context(tc.tile_pool(name="xt", bufs=3))
        persist = ctx.enter_context(tc.tile_pool(name="persist", bufs=1))
        ebuf_pool = ctx.enter_context(tc.tile_pool(name="ebuf", bufs=2))
        small = ctx.enter_context(tc.tile_pool(name="small", bufs=3))
        out_pool = ctx.enter_context(tc.tile_pool(name="outp", bufs=2))
        # PSUM banks: scores ring 2x2 + shared proj/transpose/outproj ring
        # 2x1 + attention-y ring 2x1 = exactly 8 banks
        ps_s = ctx.enter_context(tc.tile_pool(name="pss", bufs=2, space="PSUM"))
        ps_mm = ctx.enter_context(tc.tile_pool(name="psmm", bufs=2, space="PSUM"))
        ps_y = ctx.enter_context(tc.tile_pool(name="psy", bufs=2, space="PSUM"))

        xT_r = xT.rearrange("(ko p) t -> p ko t", p=128)
        out_r = out.rearrange("(tt p) c -> tt p c", p=128)

        # ---- startup DMAs. The cost model serializes all transfers on one
        # ~352 B/ns pipe, so order strictly by first use: wq/wk mt0 halves
        # (first proj groups) and xt0 lead; wp and xt1 trail.
        wq_sb = const.tile([128, KO, DG], BF16)
        wq_r = wq.rearrange("(ko p) d -> p ko d", p=128)
        wk_sb = const.tile([128, KO, DG], BF16)
        wk_r = wk.rearrange("(ko p) d -> p ko d", p=128)

        _xt_tiles = {}

        def prefetch_xt(n, eng=None, split=False):
            if n in _xt_tiles or n >= NCHUNK:
                return
            xt = xt_pool.tile([128, KO, CHUNK], BF16, tag="xt", name=f"xt{n}")
            cols = slice(n * CHUNK, (n + 1) * CHUNK)
            if split:
                nc.scalar.dma_start(xt[:, :3, :], xT_r[:, :3, cols])
                nc.gpsimd.dma_start(xt[:, 3:6, :], xT_r[:, 3:6, cols])
                nc.gpsimd.dma_start(xt[:, 6:, :], xT_r[:, 6:, cols])
            else:
                (eng or nc.gpsimd).dma_start(xt[:], xT_r[:, :, cols])
            _xt_tiles[n] = xt

        nc.sync.dma_start(wq_sb[:, :2, :], wq_r[:, :2, :])
        prefetch_xt(0, split=True)
        nc.sync.dma_start(wq_sb[:, 2:5, :], wq_r[:, 2:5, :])
        nc.sync.dma_start(wq_sb[:, 5:, :], wq_r[:, 5:, :])
        nc.sync.dma_start(wk_sb[:, :3, :], wk_r[:, :3, :])
        nc.sync.dma_start(wk_sb[:, 3:6, :], wk_r[:, 3:6, :])
        nc.sync.dma_start(wk_sb[:, 6:, :], wk_r[:, 6:, :])
        bq_sb = const.tile([128, MT], F32)
        nc.scalar.dma_start(bq_sb[:], bq.rearrange("(mt p) -> p mt", p=128))
        bk_sb = const.tile([128, MT], F32)
        nc.scalar.dma_start(bk_sb[:], bk.rearrange("(mt p) -> p mt", p=128))
        # wv and xt1 go on the same (sync) queue AFTER the wk pieces: queue
        # order keeps these big transfers from jumping ahead of wk in the
        # serialized DMA pipe, which would stall the first k-projections.
        wv_sb = const.tile([128, KO, DG], BF16)
        nc.sync.dma_start(wv_sb[:], wv.rearrange("(ko p) d -> p ko d", p=128))
        prefetch_xt(1, eng=nc.sync)
        wp_sb = const.tile([128, MT, C], BF16)
        nc.sync.dma_start(wp_sb[:], wp.rearrange("(mt p) c -> p mt c", p=128))

        ident = const.tile([128, 128], BF16)
        make_identity(nc, ident)

        # ---- persistent activations ----
        qT_sb = persist.tile([128, MT, T], BF16)     # [d_local, T] for 4 heads
        kT_sb = persist.tile([128, MT, T], BF16)
        # [tk_in, tk_tile, h, dh+1]; the last column of each head is a ones
        # column so attnT@v also accumulates the softmax denominator l.
        v_sb = persist.tile([128, NQT, HPG, DH + 1], BF16)
        nc.vector.memset(v_sb[:, :, :, DH], 1.0)

        # ---- emission units ----
        _e_tiles = {}

        def get_e(n):
            if n not in _e_tiles:
                _e_tiles[n] = ebuf_pool.tile(
                    [128, HPG, NQT, CHUNK], BF16, tag="e", name=f"e{n}"
                )
            return _e_tiles[n]

        def proj_unit(n, which, mt):
            """One (dst, mt) projection group: 8 matmuls + bias add."""
            w_sb, b_sb, dst = {
                "q": (wq_sb, bq_sb, qT_sb), "k": (wk_sb, bk_sb, kT_sb)
            }[which]
            xt = _xt_tiles[n]
            cols = slice(n * CHUNK, (n + 1) * CHUNK)
            ps = ps_mm.tile([128, CHUNK], F32, tag="mm", name=f"pj{n}{which}{mt}")
            for ko in range(KO):
                nc.tensor.matmul(
                    ps[:],
                    lhsT=w_sb[:, ko, mt * 128:(mt + 1) * 128],
                    rhs=xt[:, ko, :],
                    start=(ko == 0),
                    stop=(ko == KO - 1),
                )
            nc.vector.tensor_scalar_add(dst[:, mt, cols], ps[:], b_sb[:, mt:mt + 1])

        def vproj_unit(n, tt):
            xt = _xt_tiles[n]
            t_tile = n * QPC + tt
            ps = ps_mm.tile([128, DG], F32, tag="mm", name=f"pv{n}_{tt}")
            for ko in range(KO):
                nc.tensor.matmul(
                    ps[:],
                    lhsT=xt[:, ko, tt * 128:(tt + 1) * 128],
                    rhs=wv_sb[:, ko, :],
                    start=(ko == 0),
                    stop=(ko == KO - 1),
                )
            nc.vector.tensor_copy(v_sb[:, t_tile, :, :DH], ps[:])

        def score_step(n, m):
            """Scores + exp (+ causal zeroing) for key tile m of chunk n."""
            e_t = get_e(n)
            qlo = max(0, 128 * m - CHUNK * n)
            diag = m >= QPC * n
            for p in range(2):
                pss = ps_s.tile([128, 2, CHUNK], F32, tag="s",
                                name=f"s{n}_{m}_{p}")
                for half in range(2):
                    rows = slice(64 * half, 64 * half + 64)
                    nc.tensor.matmul(
                        pss[:, half, qlo:],
                        lhsT=kT_sb[rows, p, m * 128:(m + 1) * 128],
                        rhs=qT_sb[rows, p, n * CHUNK + qlo:(n + 1) * CHUNK],
                        start=True,
                        stop=True,
                    )
                nc.scalar.activation(
                    e_t[:, 2 * p:2 * p + 2, m, qlo:], pss[:, :, qlo:],
                    mybir.ActivationFunctionType.Exp, scale=scale,
                )
                if diag:
                    # zero sub-diagonal exp(score): keep where q - k >= 0
                    sl = e_t[:, 2 * p:2 * p + 2, m, qlo:qlo + 128]
                    nc.gpsimd.affine_select(
                        out=sl,
                        in_=sl,
                        compare_op=mybir.AluOpType.is_ge,
                        fill=0.0,
                        base=0,
                        pattern=[[0, 2], [1, 128]],
                        channel_multiplier=-1,
                    )

        _y_tiles = {}

        def attv_unit(n, qt, m_lo, m_hi):
            """y[128q, 4h, 65] accumulation for key tiles [m_lo, m_hi).

            All 4 heads share one PSUM bank: the single start zero-marks the
            whole 2KB zero region, later matmuls first-touch their own byte
            ranges."""
            e_t = get_e(n)
            tl = qt - QPC * n
            if qt not in _y_tiles:
                _y_tiles[qt] = ps_y.tile([128, HPG, 128], F32, tag="y",
                                         name=f"y{qt}")
            y = _y_tiles[qt]
            for m in range(m_lo, m_hi):
                for h in range(HPG):
                    nc.tensor.matmul(
                        y[:, h, :DH + 1],
                        lhsT=e_t[:, h, m, tl * 128:(tl + 1) * 128],
                        rhs=v_sb[:, m, h, :],
                        start=(m == 0 and h == 0),
                        stop=(m == qt and h == HPG - 1),
                    )

        _yn_tiles = {}
        _yT_tiles = {}

        def norm_unit(qt, act_assist=False):
            """Stage y to SBUF right after its accumulation stops: one copy
            is the only PSUM reader, freeing the attv ring slot for query
            tile qt+2 ~500ns earlier than recip+muls would. Normalization
            then runs off the critical path on the SBUF copy."""
            y = _y_tiles.pop(qt)
            ys = small.tile([128, HPG, DH + 1], F32, tag="ys", name=f"ys{qt}")
            if act_assist:
                nc.scalar.copy(ys[:], y[:, :, :DH + 1])
            else:
                nc.vector.tensor_copy(ys[:], y[:, :, :DH + 1])
            rc = small.tile([128, HPG], F32, tag="rc", name=f"rc{qt}")
            nc.vector.reciprocal(rc[:], ys[:, :, DH])
            y_n = small.tile([128, HPG, DH], BF16, tag="yn", name=f"yn{qt}")
            for h in range(HPG):
                nc.vector.tensor_scalar_mul(
                    y_n[:, h, :], ys[:, h, :DH], rc[:, h:h + 1]
                )
            _yn_tiles[qt] = y_n
            if qt <= CFG["dma_transp_max_qt"]:
                # yT via the DMA XBAR instead of PE transposes: issued here so
                # the ~2.4us DMA latency hides behind the filler units between
                # norm and the tail's output projection
                yT = small.tile([128, MT, 128], BF16, tag="yt", name=f"yt{qt}")
                for ks in range(MT):
                    nc.sync.dma_start_transpose(
                        yT[:, ks, :], y_n[:, 2 * ks:2 * ks + 2, :]
                    )
                _yT_tiles[qt] = yT

        def tail_transp(qt, act_assist=False):
            """Transpose phase only: PE transposes + yT copies (split across
            ACT/DVE). Used to front-load the last two tails' transposes so
            neither outproj waits on a copy queued behind other DVE work."""
            y_n = _yn_tiles.pop(qt)
            yT = small.tile([128, MT, 128], BF16, tag="yt", name=f"yt{qt}")
            for ks in range(MT):
                t = ps_mm.tile([128, 1024], BF16, tag="mm", name=f"t{qt}_{ks}")
                # one 128x128 transpose covers both heads of this ks: their
                # 64 d-columns are adjacent in y_n, and in_.T places them on
                # partition rows hh*64+d -- exactly the yT layout
                nc.tensor.transpose(
                    t[:, :128], y_n[:, 2 * ks:2 * ks + 2, :], ident[:]
                )
                if act_assist and ks == 0:
                    nc.scalar.copy(yT[:, ks, :], t[:, :128])
                else:
                    nc.vector.tensor_copy(yT[:, ks, :], t[:, :128])
            _yT_tiles[qt] = yT

        def tail_po(qt, act_assist=False, po_y=False):
            """Outproj phase for a tail whose transposes ran in tail_transp."""
            yT = _yT_tiles.pop(qt)
            o_sb = out_pool.tile([128, C], BF16, tag="o", name=f"o{qt}")
            for nh in range(2):
                if po_y:
                    ps = ps_y.tile([128, 512], F32, tag="y", name=f"po{qt}_{nh}")
                else:
                    ps = ps_mm.tile([128, 512], F32, tag="mm",
                                    name=f"po{qt}_{nh}")
                for ks in range(MT):
                    nc.tensor.matmul(
                        ps[:],
                        lhsT=yT[:, ks, :],
                        rhs=wp_sb[:, ks, nh * 512:(nh + 1) * 512],
                        start=(ks == 0),
                        stop=(ks == MT - 1),
                    )
                if act_assist and nh == 0:
                    nc.scalar.copy(o_sb[:, nh * 512:(nh + 1) * 512], ps[:])
                else:
                    nc.vector.tensor_copy(o_sb[:, nh * 512:(nh + 1) * 512], ps[:])
                nc.sync.dma_start(
                    out_r[qt][:, nh * 512:(nh + 1) * 512],
                    o_sb[:, nh * 512:(nh + 1) * 512],
                )

        def tail_unit(qt, act_assist=False, po_y=False):
            """PE-transpose -> output projection -> DMA out (per half).

            act_assist: in the final window ACT has drained its exp queue, so
            route half the PSUM->SBUF copies there to run concurrently with
            the DVE copies on the end-of-kernel critical path.
            po_y: the last two tails run after all attv accumulations have
            been normalized, so their outproj tiles can borrow the dead attv
            ring instead of choking the shared ps_mm ring."""
            y_n = _yn_tiles.pop(qt)
            if qt in _yT_tiles:
                yT = _yT_tiles.pop(qt)
            else:
                yT = small.tile([128, MT, 128], BF16, tag="yt", name=f"yt{qt}")
                for ks in range(MT):
                    t = ps_mm.tile([128, 1024], BF16, tag="mm",
                                   name=f"t{qt}_{ks}")
                    nc.tensor.transpose(
                        t[:, :128], y_n[:, 2 * ks:2 * ks + 2, :], ident[:]
                    )
                    if act_assist and ks == 0:
                        nc.scalar.copy(yT[:, ks, :], t[:, :128])
                    else:
                        nc.vector.tensor_copy(yT[:, ks, :], t[:, :128])
            o_sb = out_pool.tile([128, C], BF16, tag="o", name=f"o{qt}")
            for nh in range(2):
                if po_y:
                    ps = ps_y.tile([128, 512], F32, tag="y", name=f"po{qt}_{nh}")
                else:
                    ps = ps_mm.tile([128, 512], F32, tag="mm",
                                    name=f"po{qt}_{nh}")
                for ks in range(MT):
                    nc.tensor.matmul(
                        ps[:],
                        lhsT=yT[:, ks, :],
                        rhs=wp_sb[:, ks, nh * 512:(nh + 1) * 512],
                        start=(ks == 0),
                        stop=(ks == MT - 1),
                    )
                if act_assist and nh == 0:
                    nc.scalar.copy(o_sb[:, nh * 512:(nh + 1) * 512], ps[:])
                else:
                    nc.vector.tensor_copy(o_sb[:, nh * 512:(nh + 1) * 512], ps[:])
                nc.sync.dma_start(
                    out_r[qt][:, nh * 512:(nh + 1) * 512],
                    o_sb[:, nh * 512:(nh + 1) * 512],
                )

        # ---- schedule ----
        FILL = CFG["fill"]
        PE = 0.4167  # ns per output column at full clock

        def u(fn, ns, *args):
            return (ns, fn, args)

        def attv_units(n, qt):
            units = []
            blk = CFG["attv_blk"]
            for m_lo in range(0, qt + 1, blk):
                m_hi = min(m_lo + blk, qt + 1)
                units.append(u(attv_unit, (m_hi - m_lo) * HPG * (DH + 1) * PE,
                               n, qt, m_lo, m_hi))
            return units

        def at_units(n):
            """A/T units for chunk n: the cheap DVE normalize lands right
            after each attv (freeing its PSUM bank); the PE tail (transpose +
            outproj) trails by one query tile."""
            qts = [QPC * n + i for i in range(QPC)]
            units = []
            lag = CFG.get("tail_lag", 1)
            for i, qt in enumerate(qts):
                units += attv_units(n, qt)
                units.append(u(norm_unit, 0, qt))
                if i >= lag:
                    units.append(u(tail_unit, 2304 * PE, qts[i - lag]))
            for qt in qts[QPC - lag:]:
                units.append(u(tail_unit, 2304 * PE, qt))
            return units

        def proj_units(n):
            return [u(proj_unit, KO * CHUNK * PE, n, which, mt)
                    for which in ("q", "k") for mt in range(MT)]

        def vproj_units(n):
            return [u(vproj_unit, KO * DG * PE, n, tt) for tt in range(QPC)]

        def emit(unit):
            _, fn, args = unit
            fn(*args)

        def emit_window(s_steps, fillers):
            fi = 0
            for cm in s_steps:
                score_step(*cm)
                n, m = cm
                cols = CHUNK - max(0, 128 * m - CHUNK * n)
                budget = FILL * cols / CHUNK
                while fi < len(fillers) and budget > 0:
                    emit(fillers[fi])
                    budget -= fillers[fi][0]
                    fi += 1
            while fi < len(fillers):
                emit(fillers[fi])
                fi += 1

        b0, b1, bl = CFG["borrow_w0"], CFG["borrow_w1"], CFG["borrow_late"]

        # prologue: proj(0) ordered so scores can begin after half the groups
        for which, mt in (("q", 0), ("k", 0), ("q", 1), ("k", 1)):
            proj_unit(0, which, mt)
        emit_window(
            [(0, m) for m in range(QPC)],
            vproj_units(0) + proj_units(1),
        )

        # window 0: attn chunk 0 + scores/exp chunk 1 (+borrowed S(2))
        prefetch_xt(2, eng=nc.sync)
        emit_window(
            [(1, m) for m in range(8)] + [(2, m) for m in range(b0)],
            at_units(0) + proj_units(2) + vproj_units(1),
        )

        # window 1: attn chunk 1 + scores/exp chunk 2 (+borrowed S(3))
        prefetch_xt(3, eng=nc.sync)
        emit_window(
            [(2, m) for m in range(b0, 12)] + [(3, m) for m in range(b1)],
            at_units(1) + proj_units(3) + vproj_units(2),
        )

        # window 2: attn chunk 2 + scores/exp chunk 3 (tail steps deferred)
        emit_window(
            [(3, m) for m in range(b1, 16 - bl)],
            at_units(2) + vproj_units(3),
        )

        # window 3: attn chunk 3; deferred S(3) steps land just before the
        # attv that consumes them
        late = list(range(16 - bl, 16))
        for i, qt in enumerate((12, 13, 14, 15)):
            for m in late[:]:
                if m <= qt + CFG.get('late_lead', 1) or i == 3:
                    score_step(3, m)
                    late.remove(m)
            for unit in attv_units(3, qt):
                emit(unit)
            norm_unit(qt)
            if qt == 14:
                # transp(14) before T(13): its yT copies take ACT/DVE queue
                # priority over T(13)'s output copies, unblocking po(14)
                tail_transp(14, act_assist=True)
            if 1 <= i <= 2:
                tail_unit(qt - 1, act_assist=(qt - 1 >= 13))
        tail_transp(15, act_assist=True)
        tail_po(14, act_assist=True, po_y=True)
        tail_po(15, act_assist=True, po_y=False)

    return nc


_NC_CACHE = None


def kernel(**inputs) -> np.ndarray:
    global _NC_CACHE
    x = np.asarray(inputs["x"], np.float32)
    Wq = np.asarray(inputs["Wq"], np.float32)
    Wk = np.asarray(inputs["Wk"], np.float32)
    Wv = np.asarray(inputs["Wv"], np.float32)
    Wp = np.asarray(inputs["Wp"], np.float32)
    bq = np.asarray(inputs["bq"], np.float32)
    bk = np.asarray(inputs["bk"], np.float32)
    bv = np.asarray(inputs["bv"], np.float32)
    bp = np.asarray(inputs["bp"], np.float32)

    if _NC_CACHE is None:
        _NC_CACHE = build_kernel()
    nc = _NC_CACHE

    def b16(a):
        return np.ascontiguousarray(a).astype(ml_dtypes.bfloat16)

    in_maps = []
    for c in range(NCORES):
        b, g = divmod(c, GROUPS)
        rows = slice(g * DG, (g + 1) * DG)
        in_maps.append({
            "xT": b16(x[b].T),
            "wq": b16(Wq[rows, :].T),
            "wk": b16(Wk[rows, :].T),
            "wv": b16(Wv[rows, :].T),
            "wp": b16(Wp[:, rows].T),
            "bq": np.ascontiguousarray(bq[rows]),
            "bk": np.ascontiguousarray(bk[rows]),
        })

    res = run_bass_kernel_spmd(nc, in_maps, core_ids=list(range(NCORES)))

    result = np.zeros((B, T, C), np.float32)
    for c in range(NCORES):
        b = c // GROUPS
        result[b] += np.asarray(res.results[c]["out"], np.float32)
    result += (bv @ Wp.T + bp)[None, None, :]
    return result


# revision 54
# speedup vs baseline: 1.0111x; 1.0111x over previous
"""Causal self-attention on 8 trn2 NeuronCores.

Sharding: core c -> (batch b = c // 4, head-group g = c % 4). Each core
computes 4 of the 16 heads for one batch element and the corresponding
slice of the output projection; the host sums the 4 partial projections
per batch and adds the constant bias terms (bv @ Wp.T + bp) exactly.

All matmuls run in bf16 (fp32 PSUM accumulation). The attention@V matmul
is TRANSPOSED relative to the usual formulation: out[q,d] tiles of
[128q, 65] accumulate e_sliceT @ v per (query-tile, key-tile, head), with
a ones-column in v so column 64 collects the softmax denominator on the
same partition as the outputs -- normalization is then a per-partition
reciprocal + tensor_scalar multiply. All 4 heads of a query tile share
one PSUM bank (a single start zero-marks the whole 2KB zero region).

The schedule is software-pipelined at the CHUNK level: scores+exp for
query chunk n+1 (and a few borrowed steps of n+2) are emitted
interleaved into chunk n's attention window so the ACT engine (the only
engine with exp, ~73us of work concentrated in the late causal chunks)
runs a window ahead of the PE's attention matmuls and rarely throttles
them. Causal masking of diagonal key tiles is done by zeroing
sub-diagonal exp(score) entries with affine_select on the idle GPSIMD
engine instead of PE mask matmuls. Output is written bf16 (host sums
partials in f32), halving out-DMA bytes. Startup DMAs are ordered by
first use on explicit queues because the cost model serializes all
transfers on one ~352 B/ns pipe; the end-of-kernel tails borrow the
drained attv PSUM ring and the idle ACT engine for their copies.
"""

import numpy as np
import ml_dtypes

import concourse.bass as bass
import concourse.mybir as mybir
import concourse.tile as tile
from concourse.bass_utils import run_bass_kernel_spmd

B = 2
T = 2048
C = 1024
H = 16
DH = 64
NCORES = 8
GROUPS = 4           # head groups (tensor parallel)
HPG = H // GROUPS    # heads per group = 4
DG = HPG * DH        # head-group width = 256
CHUNK = 512          # query-block size
NCHUNK = T // CHUNK  # 4
F32 = mybir.dt.float32
BF16 = mybir.dt.bfloat16

# schedule tuning knobs
CFG = {
    "fill": 750.0,      # ns of PE filler between score steps
    "borrow_w0": 3,     # S(2) steps pulled into window 0
    "borrow_w1": 5,     # S(3) steps pulled into window 1
    "borrow_late": 2,   # S(3) tail steps pushed into window 3
    "attv_blk": 8,      # key tiles per attv emission unit
    "dma_transp_max_qt": -1,  # DMA XBAR transpose for tails (off: slower)
}


def _patch_tile_drain():
    """This walrus build lowers Drain/NOP to a CTRL with a single sync-wait
    slot; TileContext's kernel-tail drain accumulates one wait per live
    semaphore and fails codegen. Split the waits across single-wait NOPs."""
    import bass_rust
    from concourse.tile import TileContext

    def _drain_and_barrier_split(self, tick_clock, wait_clock):
        probe = self.nc.sync.nop()
        wait_clock.add_sem_waits(
            probe.ins, tile.ScopedClock({None: tick_clock.global_clock})
        )
        waits = list(probe.ins.sync_info.on_wait or [])
        probe.ins.sync_info.on_wait = []
        # distribute the final-value waits across engines; the all-engine
        # barrier below joins them before the semaphore reset
        engines = [self.nc.sync, self.nc.tensor, self.nc.vector,
                   self.nc.scalar, self.nc.gpsimd]
        for i, w in enumerate(waits):
            n = engines[i % len(engines)].nop()
            if n.ins.sync_info is None:
                n.ins.sync_info = bass_rust.SyncInfo(on_wait=[w], on_update=[])
            else:
                n.ins.sync_info.on_wait = [w]
        self.nc.sync.drain()
        self.nc.all_engine_barrier()
        assert self.sems is not None
        popped = self.nc._tile_sem_poison_stack.pop()
        assert popped is self._sem_poison
        self.nc.clear_and_free_semaphores(list(self.sems.allocated().values()))
        self.nc.all_engine_barrier()

    TileContext._drain_and_barrier = _drain_and_barrier_split

    # Same single-wait limit applies to every lowered TPB instruction (the
    # 64B formats carry one EVENTS field). Post-process the BIR JSON before
    # walrus: hoist extra semaphore waits onto same-engine NoOps.
    import json as _json

    import concourse.bass2jax as bass2jax
    import concourse.bass_utils as bass_utils

    if getattr(bass_utils.compile_bir_kernel, "_wait_split", False):
        return

    _orig_compile = bass_utils.compile_bir_kernel

    def _split_multi_waits(bir_json):
        m = _json.loads(bir_json)
        counter = 0
        changed = False
        for fn in m["functions"]:
            for blk in fn["blocks"]:
                new_insts = []
                for inst in blk["instructions"]:
                    si = inst.get("sync_info")
                    waits = (si or {}).get("on_wait") or []
                    sem_waits = [w for w in waits if w.get("sync_type") == "semaphore"]
                    if len(waits) > 1 and len(sem_waits) == len(waits):
                        changed = True
                        for w in waits[:-1]:
                            counter += 1
                            new_insts.append({
                                "name": f"I-wsplit{counter}",
                                "opcode": "NoOp",
                                "engine": inst["engine"],
                                "ins": [],
                                "outs": [],
                                "sync_info": {"on_wait": [w], "on_update": []},
                            })
                        si["on_wait"] = [waits[-1]]
                    new_insts.append(inst)
                blk["instructions"] = new_insts
        if not changed:
            return bir_json
        return _json.dumps(m).encode()

    def _compile_bir_kernel_split(bir_json, tmpdir, neff_name="file.neff"):
        return _orig_compile(_split_multi_waits(bir_json), tmpdir, neff_name=neff_name)

    _compile_bir_kernel_split._wait_split = True
    bass_utils.compile_bir_kernel = _compile_bir_kernel_split
    bass2jax.compile_bir_kernel = _compile_bir_kernel_split


def build_kernel():
    _patch_tile_drain()
    nc = bass.Bass(target_bir_lowering=False, trn_type="TRN2")

    xT = nc.dram_tensor("xT", [C, T], BF16, kind="ExternalInput")
    wq = nc.dram_tensor("wq", [C, DG], BF16, kind="ExternalInput")
    wk = nc.dram_tensor("wk", [C, DG], BF16, kind="ExternalInput")
    wv = nc.dram_tensor("wv", [C, DG], BF16, kind="ExternalInput")
    wp = nc.dram_tensor("wp", [DG, C], BF16, kind="ExternalInput")
    bq = nc.dram_tensor("bq", [DG], F32, kind="ExternalInput")
    bk = nc.dram_tensor("bk", [DG], F32, kind="ExternalInput")
    out = nc.dram_tensor("out", [T, C], BF16, kind="ExternalOutput")

    KO = C // 128            # 8 contraction subtiles for the projections
    MT = DG // 128           # 2 partition tiles for qT/kT and wp rows
    NQT = T // 128           # 16 query/key 128-tiles
    QPC = CHUNK // 128       # 4 query tiles per chunk
    scale = 1.0 / np.sqrt(DH)

    from contextlib import ExitStack

    with tile.TileContext(nc) as tc, ExitStack() as ctx:
        from concourse.masks import make_identity

        const = ctx.enter_context(tc.tile_pool(name="const", bufs=1))
        xt_pool = ctx.enter_context(tc.tile_pool(name="xt", bufs=3))
        persist = ctx.enter_context(tc.tile_pool(name="persist", bufs=1))
        ebuf_pool = ctx.enter_context(tc.tile_pool(name="ebuf", bufs=2))
        small = ctx.enter_context(tc.tile_pool(name="small", bufs=3))
        out_pool = ctx.enter_context(tc.tile_pool(name="outp", bufs=2))
        # PSUM banks: scores ring 2x2 + shared proj/transpose/outproj ring
        # 2x1 + attention-y ring 2x1 = exactly 8 banks
        ps_s = ctx.enter_context(tc.tile_pool(name="pss", bufs=2, space="PSUM"))
        ps_mm = ctx.enter_context(tc.tile_pool(name="psmm", bufs=2, space="PSUM"))
        ps_y = ctx.enter_context(tc.tile_pool(name="psy", bufs=2, space="PSUM"))

        xT_r = xT.rearrange("(ko p) t -> p ko t", p=128)
        out_r = out.rearrange("(tt p) c -> tt p c", p=128)

        # ---- startup DMAs. The cost model serializes all transfers on one
        # ~352 B/ns pipe, so order strictly by first use: wq/wk mt0 halves
        # (first proj groups) and xt0 lead; wp and xt1 trail.
        wq_sb = const.tile([128, KO, DG], BF16)
        wq_r = wq.rearrange("(ko p) d -> p ko d", p=128)
        wk_sb = const.tile([128, KO, DG], BF16)
        wk_r = wk.rearrange("(ko p) d -> p ko d", p=128)

        _xt_tiles = {}

        def prefetch_xt(n, eng=None, split=False):
            if n in _xt_tiles or n >= NCHUNK:
                return
            xt = xt_pool.tile([128, KO, CHUNK], BF16, tag="xt", name=f"xt{n}")
            cols = slice(n * CHUNK, (n + 1) * CHUNK)
            if split:
                nc.scalar.dma_start(xt[:, :3, :], xT_r[:, :3, cols])
                nc.gpsimd.dma_start(xt[:, 3:6, :], xT_r[:, 3:6, cols])
                nc.gpsimd.dma_start(xt[:, 6:, :], xT_r[:, 6:, cols])
            else:
                (eng or nc.gpsimd).dma_start(xt[:], xT_r[:, :, cols])
            _xt_tiles[n] = xt

        nc.sync.dma_start(wq_sb[:, :2, :], wq_r[:, :2, :])
        prefetch_xt(0, split=True)
        nc.sync.dma_start(wq_sb[:, 2:5, :], wq_r[:, 2:5, :])
        nc.sync.dma_start(wq_sb[:, 5:, :], wq_r[:, 5:, :])
        nc.sync.dma_start(wk_sb[:, :3, :], wk_r[:, :3, :])
        nc.sync.dma_start(wk_sb[:, 3:6, :], wk_r[:, 3:6, :])
        nc.sync.dma_start(wk_sb[:, 6:, :], wk_r[:, 6:, :])
        bq_sb = const.tile([128, MT], F32)
        nc.scalar.dma_start(bq_sb[:], bq.rearrange("(mt p) -> p mt", p=128))
        bk_sb = const.tile([128, MT], F32)
        nc.scalar.dma_start(bk_sb[:], bk.rearrange("(mt p) -> p mt", p=128))
        # wv and xt1 go on the same (sync) queue AFTER the wk pieces: queue
        # order keeps these big transfers from jumping ahead of wk in the
        # serialized DMA pipe, which would stall the first k-projections.
        wv_sb = const.tile([128, KO, DG], BF16)
        nc.sync.dma_start(wv_sb[:], wv.rearrange("(ko p) d -> p ko d", p=128))
        prefetch_xt(1, eng=nc.sync)
        wp_sb = const.tile([128, MT, C], BF16)
        nc.sync.dma_start(wp_sb[:], wp.rearrange("(mt p) c -> p mt c", p=128))

        ident = const.tile([128, 128], BF16)
        make_identity(nc, ident)

        # ---- persistent activations ----
        qT_sb = persist.tile([128, MT, T], BF16)     # [d_local, T] for 4 heads
        kT_sb = persist.tile([128, MT, T], BF16)
        # [tk_in, tk_tile, h, dh+1]; the last column of each head is a ones
        # column so attnT@v also accumulates the softmax denominator l.
        v_sb = persist.tile([128, NQT, HPG, DH + 1], BF16)
        nc.vector.memset(v_sb[:, :, :, DH], 1.0)

        # ---- emission units ----
        _e_tiles = {}

        def get_e(n):
            if n not in _e_tiles:
                _e_tiles[n] = ebuf_pool.tile(
                    [128, HPG, NQT, CHUNK], BF16, tag="e", name=f"e{n}"
                )
            return _e_tiles[n]

        def proj_unit(n, which, mt):
            """One (dst, mt) projection group: 8 matmuls + bias add."""
            w_sb, b_sb, dst = {
                "q": (wq_sb, bq_sb, qT_sb), "k": (wk_sb, bk_sb, kT_sb)
            }[which]
            xt = _xt_tiles[n]
            cols = slice(n * CHUNK, (n + 1) * CHUNK)
            ps = ps_mm.tile([128, CHUNK], F32, tag="mm", name=f"pj{n}{which}{mt}")
            for ko in range(KO):
                nc.tensor.matmul(
                    ps[:],
                    lhsT=w_sb[:, ko, mt * 128:(mt + 1) * 128],
                    rhs=xt[:, ko, :],
                    start=(ko == 0),
                    stop=(ko == KO - 1),
                )
            nc.vector.tensor_scalar_add(dst[:, mt, cols], ps[:], b_sb[:, mt:mt + 1])

        def vproj_unit(n, tt):
            xt = _xt_tiles[n]
            t_tile = n * QPC + tt
            ps = ps_mm.tile([128, DG], F32, tag="mm", name=f"pv{n}_{tt}")
            for ko in range(KO):
                nc.tensor.matmul(
                    ps[:],
                    lhsT=xt[:, ko, tt * 128:(tt + 1) * 128],
                    rhs=wv_sb[:, ko, :],
                    start=(ko == 0),
                    stop=(ko == KO - 1),
                )
            nc.vector.tensor_copy(v_sb[:, t_tile, :, :DH], ps[:])

        def score_step(n, m):
            """Scores + exp (+ causal zeroing) for key tile m of chunk n."""
            e_t = get_e(n)
            qlo = max(0, 128 * m - CHUNK * n)
            diag = m >= QPC * n
            for p in range(2):
                pss = ps_s.tile([128, 2, CHUNK], F32, tag="s",
                                name=f"s{n}_{m}_{p}")
                for half in range(2):
                    rows = slice(64 * half, 64 * half + 64)
                    nc.tensor.matmul(
                        pss[:, half, qlo:],
                        lhsT=kT_sb[rows, p, m * 128:(m + 1) * 128],
                        rhs=qT_sb[rows, p, n * CHUNK + qlo:(n + 1) * CHUNK],
                        start=True,
                        stop=True,
                    )
                nc.scalar.activation(
                    e_t[:, 2 * p:2 * p + 2, m, qlo:], pss[:, :, qlo:],
                    mybir.ActivationFunctionType.Exp, scale=scale,
                )
                if diag:
                    # zero sub-diagonal exp(score): keep where q - k >= 0
                    sl = e_t[:, 2 * p:2 * p + 2, m, qlo:qlo + 128]
                    nc.gpsimd.affine_select(
                        out=sl,
                        in_=sl,
                        compare_op=mybir.AluOpType.is_ge,
                        fill=0.0,
                        base=0,
                        pattern=[[0, 2], [1, 128]],
                        channel_multiplier=-1,
                    )

        _y_tiles = {}

        def attv_unit(n, qt, m_lo, m_hi):
            """y[128q, 4h, 65] accumulation for key tiles [m_lo, m_hi).

            All 4 heads share one PSUM bank: the single start zero-marks the
            whole 2KB zero region, later matmuls first-touch their own byte
            ranges."""
            e_t = get_e(n)
            tl = qt - QPC * n
            if qt not in _y_tiles:
                _y_tiles[qt] = ps_y.tile([128, HPG, 128], F32, tag="y",
                                         name=f"y{qt}")
            y = _y_tiles[qt]
            for m in range(m_lo, m_hi):
                for h in range(HPG):
                    nc.tensor.matmul(
                        y[:, h, :DH + 1],
                        lhsT=e_t[:, h, m, tl * 128:(tl + 1) * 128],
                        rhs=v_sb[:, m, h, :],
                        start=(m == 0 and h == 0),
                        stop=(m == qt and h == HPG - 1),
                    )

        _yn_tiles = {}
        _yT_tiles = {}

        def norm_unit(qt, act_assist=False):
            """Stage y to SBUF right after its accumulation stops: one copy
            is the only PSUM reader, freeing the attv ring slot for query
            tile qt+2 ~500ns earlier than recip+muls would. Normalization
            then runs off the critical path on the SBUF copy."""
            y = _y_tiles.pop(qt)
            ys = small.tile([128, HPG, DH + 1], F32, tag="ys", name=f"ys{qt}")
            if act_assist:
                nc.scalar.copy(ys[:], y[:, :, :DH + 1])
            else:
                nc.vector.tensor_copy(ys[:], y[:, :, :DH + 1])
            rc = small.tile([128, HPG], F32, tag="rc", name=f"rc{qt}")
            nc.vector.reciprocal(rc[:], ys[:, :, DH])
            y_n = small.tile([128, HPG, DH], BF16, tag="yn", name=f"yn{qt}")
            for h in range(HPG):
                nc.vector.tensor_scalar_mul(
                    y_n[:, h, :], ys[:, h, :DH], rc[:, h:h + 1]
                )
            _yn_tiles[qt] = y_n
            if qt <= CFG["dma_transp_max_qt"]:
                # yT via the DMA XBAR instead of PE transposes: issued here so
                # the ~2.4us DMA latency hides behind the filler units between
                # norm and the tail's output projection
                yT = small.tile([128, MT, 128], BF16, tag="yt", name=f"yt{qt}")
                for ks in range(MT):
                    nc.sync.dma_start_transpose(
                        yT[:, ks, :], y_n[:, 2 * ks:2 * ks + 2, :]
                    )
                _yT_tiles[qt] = yT

        def tail_transp(qt, act_assist=False):
            """Transpose phase only: PE transposes + yT copies (split across
            ACT/DVE). Used to front-load the last two tails' transposes so
            neither outproj waits on a copy queued behind other DVE work."""
            y_n = _yn_tiles.pop(qt)
            yT = small.tile([128, MT, 128], BF16, tag="yt", name=f"yt{qt}")
            for ks in range(MT):
                t = ps_mm.tile([128, 1024], BF16, tag="mm", name=f"t{qt}_{ks}")
                # one 128x128 transpose covers both heads of this ks: their
                # 64 d-columns are adjacent in y_n, and in_.T places them on
                # partition rows hh*64+d -- exactly the yT layout
                nc.tensor.transpose(
                    t[:, :128], y_n[:, 2 * ks:2 * ks + 2, :], ident[:]
                )
                if act_assist and ks == 0:
                    nc.scalar.copy(yT[:, ks, :], t[:, :128])
                else:
                    nc.vector.tensor_copy(yT[:, ks, :], t[:, :128])
            _yT_tiles[qt] = yT

        def tail_po(qt, act_assist=False, po_y=False):
            """Outproj phase for a tail whose transposes ran in tail_transp."""
            yT = _yT_tiles.pop(qt)
            o_sb = out_pool.tile([128, C], BF16, tag="o", name=f"o{qt}")
            for nh in range(2):
                if po_y:
                    ps = ps_y.tile([128, 512], F32, tag="y", name=f"po{qt}_{nh}")
                else:
                    ps = ps_mm.tile([128, 512], F32, tag="mm",
                                    name=f"po{qt}_{nh}")
                for ks in range(MT):
                    nc.tensor.matmul(
                        ps[:],
                        lhsT=yT[:, ks, :],
                        rhs=wp_sb[:, ks, nh * 512:(nh + 1) * 512],
                        start=(ks == 0),
                        stop=(ks == MT - 1),
                    )
                if act_assist and nh == 0:
                    nc.scalar.copy(o_sb[:, nh * 512:(nh + 1) * 512], ps[:])
                else:
                    nc.vector.tensor_copy(o_sb[:, nh * 512:(nh + 1) * 512], ps[:])
                nc.sync.dma_start(
                    out_r[qt][:, nh * 512:(nh + 1) * 512],
                    o_sb[:, nh * 512:(nh + 1) * 512],
                )

        def tail_unit(qt, act_assist=False, po_y=False):
            """PE-transpose -> output projection -> DMA out (per half).

            act_assist: in the final window ACT has drained its exp queue, so
            route half the PSUM->SBUF copies there to run concurrently with
            the DVE copies on the end-of-kernel critical path.
            po_y: the last two tails run after all attv accumulations have
            been normalized, so their outproj tiles can borrow the dead attv
            ring instead of choking the shared ps_mm ring."""
            y_n = _yn_tiles.pop(qt)
            if qt in _yT_tiles:
                yT = _yT_tiles.pop(qt)
            else:
                yT = small.tile([128, MT, 128], BF16, tag="yt", name=f"yt{qt}")
                for ks in range(MT):
                    t = ps_mm.tile([128, 1024], BF16, tag="mm",
                                   name=f"t{qt}_{ks}")
                    nc.tensor.transpose(
                        t[:, :128], y_n[:, 2 * ks:2 * ks + 2, :], ident[:]
                    )
                    if act_assist and ks == 0:
                        nc.scalar.copy(yT[:, ks, :], t[:, :128])
                    else:
                        nc.vector.tensor_copy(yT[:, ks, :], t[:, :128])
            o_sb = out_pool.tile([128, C], BF16, tag="o", name=f"o{qt}")
            for nh in range(2):
                if po_y:
                    ps = ps_y.tile([128, 512], F32, tag="y", name=f"po{qt}_{nh}")
                else:
                    ps = ps_mm.tile([128, 512], F32, tag="mm",
                                    name=f"po{qt}_{nh}")
                for ks in range(MT):
                    nc.tensor.matmul(
                        ps[:],
                        lhsT=yT[:, ks, :],
                        rhs=wp_sb[:, ks, nh * 512:(nh + 1) * 512],
                        start=(ks == 0),
                        stop=(ks == MT - 1),
                    )
                if act_assist and nh == 0:
                    nc.scalar.copy(o_sb[:, nh * 512:(nh + 1) * 512], ps[:])
                else:
                    nc.vector.tensor_copy(o_sb[:, nh * 512:(nh + 1) * 512], ps[:])
                nc.sync.dma_start(
                    out_r[qt][:, nh * 512:(nh + 1) * 512],
                    o_sb[:, nh * 512:(nh + 1) * 512],
                )

        # ---- schedule ----
        FILL = CFG["fill"]
        PE = 0.4167  # ns per output column at full clock

        def u(fn, ns, *args):
            return (ns, fn, args)

        def attv_units(n, qt):
            units = []
            blk = CFG["attv_blk"]
            for m_lo in range(0, qt + 1, blk):
                m_hi = min(m_lo + blk, qt + 1)
                units.append(u(attv_unit, (m_hi - m_lo) * HPG * (DH + 1) * PE,
                               n, qt, m_lo, m_hi))
            return units

        def at_units(n):
            """A/T units for chunk n: the cheap DVE normalize lands right
            after each attv (freeing its PSUM bank); the PE tail (transpose +
            outproj) trails by one query tile."""
            qts = [QPC * n + i for i in range(QPC)]
            units = []
            lag = CFG.get("tail_lag", 1)
            for i, qt in enumerate(qts):
                units += attv_units(n, qt)
                units.append(u(norm_unit, 0, qt))
                if i >= lag:
                    units.append(u(tail_unit, 2304 * PE, qts[i - lag]))
            for qt in qts[QPC - lag:]:
                units.append(u(tail_unit, 2304 * PE, qt))
            return units

        def proj_units(n):
            return [u(proj_unit, KO * CHUNK * PE, n, which, mt)
                    for which in ("q", "k") for mt in range(MT)]

        def vproj_units(n):
            return [u(vproj_unit, KO * DG * PE, n, tt) for tt in range(QPC)]

        def emit(unit):
            _, fn, args = unit
            fn(*args)

        def emit_window(s_steps, fillers):
            fi = 0
            for cm in s_steps:
                score_step(*cm)
                n, m = cm
                cols = CHUNK - max(0, 128 * m - CHUNK * n)
                budget = FILL * cols / CHUNK
                while fi < len(fillers) and budget > 0:
                    emit(fillers[fi])
                    budget -= fillers[fi][0]
                    fi += 1
            while fi < len(fillers):
                emit(fillers[fi])
                fi += 1

        b0, b1, bl = CFG["borrow_w0"], CFG["borrow_w1"], CFG["borrow_late"]

        # prologue: proj(0) ordered so scores can begin after half the groups
        for which, mt in (("q", 0), ("k", 0), ("q", 1), ("k", 1)):
            proj_unit(0, which, mt)
        emit_window(
            [(0, m) for m in range(QPC)],
            vproj_units(0) + proj_units(1),
        )

        # window 0: attn chunk 0 + scores/exp chunk 1 (+borrowed S(2))
        prefetch_xt(2, eng=nc.sync)
        emit_window(
            [(1, m) for m in range(8)] + [(2, m) for m in range(b0)],
            at_units(0) + proj_units(2) + vproj_units(1),
        )

        # window 1: attn chunk 1 + scores/exp chunk 2 (+borrowed S(3))
        prefetch_xt(3, eng=nc.sync)
        emit_window(
            [(2, m) for m in range(b0, 12)] + [(3, m) for m in range(b1)],
            at_units(1) + proj_units(3) + vproj_units(2),
        )

        # window 2: attn chunk 2 + scores/exp chunk 3 (tail steps deferred)
        emit_window(
            [(3, m) for m in range(b1, 16 - bl)],
            at_units(2) + vproj_units(3),
        )

        # window 3: attn chunk 3; deferred S(3) steps land just before the
        # attv that consumes them
        late = list(range(16 - bl, 16))
        for i, qt in enumerate((12, 13, 14, 15)):
            for m in late[:]:
                if m <= qt + CFG.get('late_lead', 1) or i == 3:
                    score_step(3, m)
                    late.remove(m)
            for unit in attv_units(3, qt):
                emit(unit)
            norm_unit(qt)
            if 1 <= i <= 2:
                tail_unit(qt - 1, act_assist=(qt - 1 >= 13))
        tail_transp(14, act_assist=True)
        tail_transp(15, act_assist=True)
        tail_po(14, act_assist=True, po_y=True)
        tail_po(15, act_assist=True, po_y=False)

    return nc


_NC_CACHE = None


def kernel(**inputs) -> np.ndarray:
    global _NC_CACHE
    x = np.asarray(inputs["x"], np.float32)
    Wq = np.asarray(inputs["Wq"], np.float32)
    Wk = np.asarray(inputs["Wk"], np.float32)
    Wv = np.asarray(inputs["Wv"], np.float32)
    Wp = np.asarray(inputs["Wp"], np.float32)
    bq = np.asarray(inputs["bq"], np.float32)
    bk = np.asarray(inputs["bk"], np.float32)
    bv = np.asarray(inputs["bv"], np.float32)
    bp = np.asarray(inputs["bp"], np.float32)

    if _NC_CACHE is None:
        _NC_CACHE = build_kernel()
    nc = _NC_CACHE

    def b16(a):
        return np.ascontiguousarray(a).astype(ml_dtypes.bfloat16)

    in_maps = []
    for c in range(NCORES):
        b, g = divmod(c, GROUPS)
        rows = slice(g * DG, (g + 1) * DG)
        in_maps.append({
            "xT": b16(x[b].T),
            "wq": b16(Wq[rows, :].T),
            "wk": b16(Wk[rows, :].T),
            "wv": b16(Wv[rows, :].T),
            "wp": b16(Wp[:, rows].T),
            "bq": np.ascontiguousarray(bq[rows]),
            "bk": np.ascontiguousarray(bk[rows]),
        })

    res = run_bass_kernel_spmd(nc, in_maps, core_ids=list(range(NCORES)))

    result = np.zeros((B, T, C), np.float32)
    for c in range(NCORES):
        b = c // GROUPS
        result[b] += np.asarray(res.results[c]["out"], np.float32)
    result += (bv @ Wp.T + bp)[None, None, :]
    return result


# revision 55
# speedup vs baseline: 1.0113x; 1.0002x over previous
"""Causal self-attention on 8 trn2 NeuronCores.

Sharding: core c -> (batch b = c // 4, head-group g = c % 4). Each core
computes 4 of the 16 heads for one batch element and the corresponding
slice of the output projection; the host sums the 4 partial projections
per batch and adds the constant bias terms (bv @ Wp.T + bp) exactly.

All matmuls run in bf16 (fp32 PSUM accumulation). The attention@V matmul
is TRANSPOSED relative to the usual formulation: out[q,d] tiles of
[128q, 65] accumulate e_sliceT @ v per (query-tile, key-tile, head), with
a ones-column in v so column 64 collects the softmax denominator on the
same partition as the outputs -- normalization is then a per-partition
reciprocal + tensor_scalar multiply. All 4 heads of a query tile share
one PSUM bank (a single start zero-marks the whole 2KB zero region).

The schedule is software-pipelined at the CHUNK level: scores+exp for
query chunk n+1 (and a few borrowed steps of n+2) are emitted
interleaved into chunk n's attention window so the ACT engine (the only
engine with exp, ~73us of work concentrated in the late causal chunks)
runs a window ahead of the PE's attention matmuls and rarely throttles
them. Causal masking of diagonal key tiles is done by zeroing
sub-diagonal exp(score) entries with affine_select on the idle GPSIMD
engine instead of PE mask matmuls. Output is written bf16 (host sums
partials in f32), halving out-DMA bytes. Startup DMAs are ordered by
first use on explicit queues because the cost model serializes all
transfers on one ~352 B/ns pipe; the end-of-kernel tails borrow the
drained attv PSUM ring and the idle ACT engine for their copies.
"""

import numpy as np
import ml_dtypes

import concourse.bass as bass
import concourse.mybir as mybir
import concourse.tile as tile
from concourse.bass_utils import run_bass_kernel_spmd

B = 2
T = 2048
C = 1024
H = 16
DH = 64
NCORES = 8
GROUPS = 4           # head groups (tensor parallel)
HPG = H // GROUPS    # heads per group = 4
DG = HPG * DH        # head-group width = 256
CHUNK = 512          # query-block size
NCHUNK = T // CHUNK  # 4
F32 = mybir.dt.float32
BF16 = mybir.dt.bfloat16

# schedule tuning knobs
CFG = {
    "fill": 750.0,      # ns of PE filler between score steps
    "borrow_w0": 3,     # S(2) steps pulled into window 0
    "borrow_w1": 5,     # S(3) steps pulled into window 1
    "borrow_late": 2,   # S(3) tail steps pushed into window 3
    "attv_blk": 8,      # key tiles per attv emission unit
    "dma_transp_max_qt": -1,  # DMA XBAR transpose for tails (off: slower)
}


def _patch_tile_drain():
    """This walrus build lowers Drain/NOP to a CTRL with a single sync-wait
    slot; TileContext's kernel-tail drain accumulates one wait per live
    semaphore and fails codegen. Split the waits across single-wait NOPs."""
    import bass_rust
    from concourse.tile import TileContext

    def _drain_and_barrier_split(self, tick_clock, wait_clock):
        probe = self.nc.sync.nop()
        wait_clock.add_sem_waits(
            probe.ins, tile.ScopedClock({None: tick_clock.global_clock})
        )
        waits = list(probe.ins.sync_info.on_wait or [])
        probe.ins.sync_info.on_wait = []
        # distribute the final-value waits across engines; the all-engine
        # barrier below joins them before the semaphore reset
        engines = [self.nc.sync, self.nc.tensor, self.nc.vector,
                   self.nc.scalar, self.nc.gpsimd]
        for i, w in enumerate(waits):
            n = engines[i % len(engines)].nop()
            if n.ins.sync_info is None:
                n.ins.sync_info = bass_rust.SyncInfo(on_wait=[w], on_update=[])
            else:
                n.ins.sync_info.on_wait = [w]
        self.nc.sync.drain()
        self.nc.all_engine_barrier()
        assert self.sems is not None
        popped = self.nc._tile_sem_poison_stack.pop()
        assert popped is self._sem_poison
        self.nc.clear_and_free_semaphores(list(self.sems.allocated().values()))
        self.nc.all_engine_barrier()

    TileContext._drain_and_barrier = _drain_and_barrier_split

    # Same single-wait limit applies to every lowered TPB instruction (the
    # 64B formats carry one EVENTS field). Post-process the BIR JSON before
    # walrus: hoist extra semaphore waits onto same-engine NoOps.
    import json as _json

    import concourse.bass2jax as bass2jax
    import concourse.bass_utils as bass_utils

    if getattr(bass_utils.compile_bir_kernel, "_wait_split", False):
        return

    _orig_compile = bass_utils.compile_bir_kernel

    def _split_multi_waits(bir_json):
        m = _json.loads(bir_json)
        counter = 0
        changed = False
        for fn in m["functions"]:
            for blk in fn["blocks"]:
                new_insts = []
                for inst in blk["instructions"]:
                    si = inst.get("sync_info")
                    waits = (si or {}).get("on_wait") or []
                    sem_waits = [w for w in waits if w.get("sync_type") == "semaphore"]
                    if len(waits) > 1 and len(sem_waits) == len(waits):
                        changed = True
                        for w in waits[:-1]:
                            counter += 1
                            new_insts.append({
                                "name": f"I-wsplit{counter}",
                                "opcode": "NoOp",
                                "engine": inst["engine"],
                                "ins": [],
                                "outs": [],
                                "sync_info": {"on_wait": [w], "on_update": []},
                            })
                        si["on_wait"] = [waits[-1]]
                    new_insts.append(inst)
                blk["instructions"] = new_insts
        if not changed:
            return bir_json
        return _json.dumps(m).encode()

    def _compile_bir_kernel_split(bir_json, tmpdir, neff_name="file.neff"):
        return _orig_compile(_split_multi_waits(bir_json), tmpdir, neff_name=neff_name)

    _compile_bir_kernel_split._wait_split = True
    bass_utils.compile_bir_kernel = _compile_bir_kernel_split
    bass2jax.compile_bir_kernel = _compile_bir_kernel_split


def build_kernel():
    _patch_tile_drain()
    nc = bass.Bass(target_bir_lowering=False, trn_type="TRN2")

    xT = nc.dram_tensor("xT", [C, T], BF16, kind="ExternalInput")
    wq = nc.dram_tensor("wq", [C, DG], BF16, kind="ExternalInput")
    wk = nc.dram_tensor("wk", [C, DG], BF16, kind="ExternalInput")
    wv = nc.dram_tensor("wv", [C, DG], BF16, kind="ExternalInput")
    wp = nc.dram_tensor("wp", [DG, C], BF16, kind="ExternalInput")
    bq = nc.dram_tensor("bq", [DG], F32, kind="ExternalInput")
    bk = nc.dram_tensor("bk", [DG], F32, kind="ExternalInput")
    out = nc.dram_tensor("out", [T, C], BF16, kind="ExternalOutput")

    KO = C // 128            # 8 contraction subtiles for the projections
    MT = DG // 128           # 2 partition tiles for qT/kT and wp rows
    NQT = T // 128           # 16 query/key 128-tiles
    QPC = CHUNK // 128       # 4 query tiles per chunk
    scale = 1.0 / np.sqrt(DH)

    from contextlib import ExitStack

    with tile.TileContext(nc) as tc, ExitStack() as ctx:
        from concourse.masks import make_identity

        const = ctx.enter_context(tc.tile_pool(name="const", bufs=1))
        xt_pool = ctx.enter_context(tc.tile_pool(name="xt", bufs=3))
        persist = ctx.enter_context(tc.tile_pool(name="persist", bufs=1))
        ebuf_pool = ctx.enter_context(tc.tile_pool(name="ebuf", bufs=2))
        small = ctx.enter_context(tc.tile_pool(name="small", bufs=3))
        out_pool = ctx.enter_context(tc.tile_pool(name="outp", bufs=2))
        # PSUM banks: scores ring 2x2 + shared proj/transpose/outproj ring
        # 2x1 + attention-y ring 2x1 = exactly 8 banks
        ps_s = ctx.enter_context(tc.tile_pool(name="pss", bufs=2, space="PSUM"))
        ps_mm = ctx.enter_context(tc.tile_pool(name="psmm", bufs=2, space="PSUM"))
        ps_y = ctx.enter_context(tc.tile_pool(name="psy", bufs=2, space="PSUM"))

        xT_r = xT.rearrange("(ko p) t -> p ko t", p=128)
        out_r = out.rearrange("(tt p) c -> tt p c", p=128)

        # ---- startup DMAs. The cost model serializes all transfers on one
        # ~352 B/ns pipe, so order strictly by first use: wq/wk mt0 halves
        # (first proj groups) and xt0 lead; wp and xt1 trail.
        wq_sb = const.tile([128, KO, DG], BF16)
        wq_r = wq.rearrange("(ko p) d -> p ko d", p=128)
        wk_sb = const.tile([128, KO, DG], BF16)
        wk_r = wk.rearrange("(ko p) d -> p ko d", p=128)

        _xt_tiles = {}

        def prefetch_xt(n, eng=None, split=False):
            if n in _xt_tiles or n >= NCHUNK:
                return
            xt = xt_pool.tile([128, KO, CHUNK], BF16, tag="xt", name=f"xt{n}")
            cols = slice(n * CHUNK, (n + 1) * CHUNK)
            if split:
                nc.scalar.dma_start(xt[:, :3, :], xT_r[:, :3, cols])
                nc.gpsimd.dma_start(xt[:, 3:6, :], xT_r[:, 3:6, cols])
                nc.gpsimd.dma_start(xt[:, 6:, :], xT_r[:, 6:, cols])
            else:
                (eng or nc.gpsimd).dma_start(xt[:], xT_r[:, :, cols])
            _xt_tiles[n] = xt

        nc.sync.dma_start(wq_sb[:, :2, :], wq_r[:, :2, :])
        prefetch_xt(0, split=True)
        nc.sync.dma_start(wq_sb[:, 2:5, :], wq_r[:, 2:5, :])
        nc.sync.dma_start(wq_sb[:, 5:, :], wq_r[:, 5:, :])
        nc.sync.dma_start(wk_sb[:, :3, :], wk_r[:, :3, :])
        nc.sync.dma_start(wk_sb[:, 3:6, :], wk_r[:, 3:6, :])
        nc.sync.dma_start(wk_sb[:, 6:, :], wk_r[:, 6:, :])
        bq_sb = const.tile([128, MT], F32)
        nc.scalar.dma_start(bq_sb[:], bq.rearrange("(mt p) -> p mt", p=128))
        bk_sb = const.tile([128, MT], F32)
        nc.scalar.dma_start(bk_sb[:], bk.rearrange("(mt p) -> p mt", p=128))
        # wv and xt1 go on the same (sync) queue AFTER the wk pieces: queue
        # order keeps these big transfers from jumping ahead of wk in the
        # serialized DMA pipe, which would stall the first k-projections.
        wv_sb = const.tile([128, KO, DG], BF16)
        nc.sync.dma_start(wv_sb[:], wv.rearrange("(ko p) d -> p ko d", p=128))
        prefetch_xt(1, eng=nc.sync)
        wp_sb = const.tile([128, MT, C], BF16)
        nc.sync.dma_start(wp_sb[:], wp.rearrange("(mt p) c -> p mt c", p=128))

        ident = const.tile([128, 128], BF16)
        make_identity(nc, ident)

        # ---- persistent activations ----
        qT_sb = persist.tile([128, MT, T], BF16)     # [d_local, T] for 4 heads
        kT_sb = persist.tile([128, MT, T], BF16)
        # [tk_in, tk_tile, h, dh+1]; the last column of each head is a ones
        # column so attnT@v also accumulates the softmax denominator l.
        v_sb = persist.tile([128, NQT, HPG, DH + 1], BF16)
        nc.vector.memset(v_sb[:, :, :, DH], 1.0)

        # ---- emission units ----
        _e_tiles = {}

        def get_e(n):
            if n not in _e_tiles:
                _e_tiles[n] = ebuf_pool.tile(
                    [128, HPG, NQT, CHUNK], BF16, tag="e", name=f"e{n}"
                )
            return _e_tiles[n]

        def proj_unit(n, which, mt):
            """One (dst, mt) projection group: 8 matmuls + bias add."""
            w_sb, b_sb, dst = {
                "q": (wq_sb, bq_sb, qT_sb), "k": (wk_sb, bk_sb, kT_sb)
            }[which]
            xt = _xt_tiles[n]
            cols = slice(n * CHUNK, (n + 1) * CHUNK)
            ps = ps_mm.tile([128, CHUNK], F32, tag="mm", name=f"pj{n}{which}{mt}")
            for ko in range(KO):
                nc.tensor.matmul(
                    ps[:],
                    lhsT=w_sb[:, ko, mt * 128:(mt + 1) * 128],
                    rhs=xt[:, ko, :],
                    start=(ko == 0),
                    stop=(ko == KO - 1),
                )
            nc.vector.tensor_scalar_add(dst[:, mt, cols], ps[:], b_sb[:, mt:mt + 1])

        def vproj_unit(n, tt):
            xt = _xt_tiles[n]
            t_tile = n * QPC + tt
            ps = ps_mm.tile([128, DG], F32, tag="mm", name=f"pv{n}_{tt}")
            for ko in range(KO):
                nc.tensor.matmul(
                    ps[:],
                    lhsT=xt[:, ko, tt * 128:(tt + 1) * 128],
                    rhs=wv_sb[:, ko, :],
                    start=(ko == 0),
                    stop=(ko == KO - 1),
                )
            nc.vector.tensor_copy(v_sb[:, t_tile, :, :DH], ps[:])

        def score_step(n, m):
            """Scores + exp (+ causal zeroing) for key tile m of chunk n."""
            e_t = get_e(n)
            qlo = max(0, 128 * m - CHUNK * n)
            diag = m >= QPC * n
            for p in range(2):
                pss = ps_s.tile([128, 2, CHUNK], F32, tag="s",
                                name=f"s{n}_{m}_{p}")
                for half in range(2):
                    rows = slice(64 * half, 64 * half + 64)
                    nc.tensor.matmul(
                        pss[:, half, qlo:],
                        lhsT=kT_sb[rows, p, m * 128:(m + 1) * 128],
                        rhs=qT_sb[rows, p, n * CHUNK + qlo:(n + 1) * CHUNK],
                        start=True,
                        stop=True,
                    )
                nc.scalar.activation(
                    e_t[:, 2 * p:2 * p + 2, m, qlo:], pss[:, :, qlo:],
                    mybir.ActivationFunctionType.Exp, scale=scale,
                )
                if diag:
                    # zero sub-diagonal exp(score): keep where q - k >= 0
                    sl = e_t[:, 2 * p:2 * p + 2, m, qlo:qlo + 128]
                    nc.gpsimd.affine_select(
                        out=sl,
                        in_=sl,
                        compare_op=mybir.AluOpType.is_ge,
                        fill=0.0,
                        base=0,
                        pattern=[[0, 2], [1, 128]],
                        channel_multiplier=-1,
                    )

        _y_tiles = {}

        def attv_unit(n, qt, m_lo, m_hi):
            """y[128q, 4h, 65] accumulation for key tiles [m_lo, m_hi).

            All 4 heads share one PSUM bank: the single start zero-marks the
            whole 2KB zero region, later matmuls first-touch their own byte
            ranges."""
            e_t = get_e(n)
            tl = qt - QPC * n
            if qt not in _y_tiles:
                _y_tiles[qt] = ps_y.tile([128, HPG, 128], F32, tag="y",
                                         name=f"y{qt}")
            y = _y_tiles[qt]
            for m in range(m_lo, m_hi):
                for h in range(HPG):
                    nc.tensor.matmul(
                        y[:, h, :DH + 1],
                        lhsT=e_t[:, h, m, tl * 128:(tl + 1) * 128],
                        rhs=v_sb[:, m, h, :],
                        start=(m == 0 and h == 0),
                        stop=(m == qt and h == HPG - 1),
                    )

        _yn_tiles = {}
        _yT_tiles = {}

        def norm_unit(qt, act_assist=False):
            """Stage y to SBUF right after its accumulation stops: one copy
            is the only PSUM reader, freeing the attv ring slot for query
            tile qt+2 ~500ns earlier than recip+muls would. Normalization
            then runs off the critical path on the SBUF copy."""
            y = _y_tiles.pop(qt)
            ys = small.tile([128, HPG, DH + 1], F32, tag="ys", name=f"ys{qt}")
            if act_assist:
                nc.scalar.copy(ys[:], y[:, :, :DH + 1])
            else:
                nc.vector.tensor_copy(ys[:], y[:, :, :DH + 1])
            rc = small.tile([128, HPG], F32, tag="rc", name=f"rc{qt}")
            nc.vector.reciprocal(rc[:], ys[:, :, DH])
            y_n = small.tile([128, HPG, DH], BF16, tag="yn", name=f"yn{qt}")
            for h in range(HPG):
                nc.vector.tensor_scalar_mul(
                    y_n[:, h, :], ys[:, h, :DH], rc[:, h:h + 1]
                )
            _yn_tiles[qt] = y_n
            if qt <= CFG["dma_transp_max_qt"]:
                # yT via the DMA XBAR instead of PE transposes: issued here so
                # the ~2.4us DMA latency hides behind the filler units between
                # norm and the tail's output projection
                yT = small.tile([128, MT, 128], BF16, tag="yt", name=f"yt{qt}")
                for ks in range(MT):
                    nc.sync.dma_start_transpose(
                        yT[:, ks, :], y_n[:, 2 * ks:2 * ks + 2, :]
                    )
                _yT_tiles[qt] = yT

        def tail_transp(qt, act_assist=False):
            """Transpose phase only: PE transposes + yT copies (split across
            ACT/DVE). Used to front-load the last two tails' transposes so
            neither outproj waits on a copy queued behind other DVE work."""
            y_n = _yn_tiles.pop(qt)
            yT = small.tile([128, MT, 128], BF16, tag="yt", name=f"yt{qt}")
            for ks in range(MT):
                t = ps_mm.tile([128, 1024], BF16, tag="mm", name=f"t{qt}_{ks}")
                # one 128x128 transpose covers both heads of this ks: their
                # 64 d-columns are adjacent in y_n, and in_.T places them on
                # partition rows hh*64+d -- exactly the yT layout
                nc.tensor.transpose(
                    t[:, :128], y_n[:, 2 * ks:2 * ks + 2, :], ident[:]
                )
                if act_assist and ks == 0:
                    nc.scalar.copy(yT[:, ks, :], t[:, :128])
                else:
                    nc.vector.tensor_copy(yT[:, ks, :], t[:, :128])
            _yT_tiles[qt] = yT

        def tail_po(qt, act_assist=False, po_y=False):
            """Outproj phase for a tail whose transposes ran in tail_transp."""
            yT = _yT_tiles.pop(qt)
            o_sb = out_pool.tile([128, C], BF16, tag="o", name=f"o{qt}")
            for nh in range(2):
                if po_y:
                    ps = ps_y.tile([128, 512], F32, tag="y", name=f"po{qt}_{nh}")
                else:
                    ps = ps_mm.tile([128, 512], F32, tag="mm",
                                    name=f"po{qt}_{nh}")
                for ks in range(MT):
                    nc.tensor.matmul(
                        ps[:],
                        lhsT=yT[:, ks, :],
                        rhs=wp_sb[:, ks, nh * 512:(nh + 1) * 512],
                        start=(ks == 0),
                        stop=(ks == MT - 1),
                    )
                if act_assist and nh == 0:
                    nc.scalar.copy(o_sb[:, nh * 512:(nh + 1) * 512], ps[:])
                else:
                    nc.vector.tensor_copy(o_sb[:, nh * 512:(nh + 1) * 512], ps[:])
                nc.sync.dma_start(
                    out_r[qt][:, nh * 512:(nh + 1) * 512],
                    o_sb[:, nh * 512:(nh + 1) * 512],
                )

        def tail_unit(qt, act_assist=False, po_y=False):
            """PE-transpose -> output projection -> DMA out (per half).

            act_assist: in the final window ACT has drained its exp queue, so
            route half the PSUM->SBUF copies there to run concurrently with
            the DVE copies on the end-of-kernel critical path.
            po_y: the last two tails run after all attv accumulations have
            been normalized, so their outproj tiles can borrow the dead attv
            ring instead of choking the shared ps_mm ring."""
            y_n = _yn_tiles.pop(qt)
            if qt in _yT_tiles:
                yT = _yT_tiles.pop(qt)
            else:
                yT = small.tile([128, MT, 128], BF16, tag="yt", name=f"yt{qt}")
                for ks in range(MT):
                    t = ps_mm.tile([128, 1024], BF16, tag="mm",
                                   name=f"t{qt}_{ks}")
                    nc.tensor.transpose(
                        t[:, :128], y_n[:, 2 * ks:2 * ks + 2, :], ident[:]
                    )
                    if act_assist and ks == 0:
                        nc.scalar.copy(yT[:, ks, :], t[:, :128])
                    else:
                        nc.vector.tensor_copy(yT[:, ks, :], t[:, :128])
            o_sb = out_pool.tile([128, C], BF16, tag="o", name=f"o{qt}")
            for nh in range(2):
                if po_y:
                    ps = ps_y.tile([128, 512], F32, tag="y", name=f"po{qt}_{nh}")
                else:
                    ps = ps_mm.tile([128, 512], F32, tag="mm",
                                    name=f"po{qt}_{nh}")
                for ks in range(MT):
                    nc.tensor.matmul(
                        ps[:],
                        lhsT=yT[:, ks, :],
                        rhs=wp_sb[:, ks, nh * 512:(nh + 1) * 512],
                        start=(ks == 0),
                        stop=(ks == MT - 1),
                    )
                if act_assist and nh == 0:
                    nc.scalar.copy(o_sb[:, nh * 512:(nh + 1) * 512], ps[:])
                else:
                    nc.vector.tensor_copy(o_sb[:, nh * 512:(nh + 1) * 512], ps[:])
                nc.sync.dma_start(
                    out_r[qt][:, nh * 512:(nh + 1) * 512],
                    o_sb[:, nh * 512:(nh + 1) * 512],
                )

        # ---- schedule ----
        FILL = CFG["fill"]
        PE = 0.4167  # ns per output column at full clock

        def u(fn, ns, *args):
            return (ns, fn, args)

        def attv_units(n, qt):
            units = []
            blk = CFG["attv_blk"]
            for m_lo in range(0, qt + 1, blk):
                m_hi = min(m_lo + blk, qt + 1)
                units.append(u(attv_unit, (m_hi - m_lo) * HPG * (DH + 1) * PE,
                               n, qt, m_lo, m_hi))
            return units

        def at_units(n):
            """A/T units for chunk n: the cheap DVE normalize lands right
            after each attv (freeing its PSUM bank); the PE tail (transpose +
            outproj) trails by one query tile."""
            qts = [QPC * n + i for i in range(QPC)]
            units = []
            lag = CFG.get("tail_lag", 1)
            for i, qt in enumerate(qts):
                units += attv_units(n, qt)
                units.append(u(norm_unit, 0, qt))
                if i >= lag:
                    units.append(u(tail_unit, 2304 * PE, qts[i - lag]))
            for qt in qts[QPC - lag:]:
                units.append(u(tail_unit, 2304 * PE, qt))
            return units

        def proj_units(n):
            return [u(proj_unit, KO * CHUNK * PE, n, which, mt)
                    for which in ("q", "k") for mt in range(MT)]

        def vproj_units(n):
            return [u(vproj_unit, KO * DG * PE, n, tt) for tt in range(QPC)]

        def emit(unit):
            _, fn, args = unit
            fn(*args)

        def emit_window(s_steps, fillers):
            fi = 0
            for cm in s_steps:
                score_step(*cm)
                n, m = cm
                cols = CHUNK - max(0, 128 * m - CHUNK * n)
                budget = FILL * cols / CHUNK
                while fi < len(fillers) and budget > 0:
                    emit(fillers[fi])
                    budget -= fillers[fi][0]
                    fi += 1
            while fi < len(fillers):
                emit(fillers[fi])
                fi += 1

        b0, b1, bl = CFG["borrow_w0"], CFG["borrow_w1"], CFG["borrow_late"]

        # prologue: proj(0) ordered so scores can begin after half the groups
        for which, mt in (("q", 0), ("q", 1), ("k", 0), ("k", 1)):
            proj_unit(0, which, mt)
        emit_window(
            [(0, m) for m in range(QPC)],
            vproj_units(0) + proj_units(1),
        )

        # window 0: attn chunk 0 + scores/exp chunk 1 (+borrowed S(2))
        prefetch_xt(2, eng=nc.sync)
        emit_window(
            [(1, m) for m in range(8)] + [(2, m) for m in range(b0)],
            at_units(0) + proj_units(2) + vproj_units(1),
        )

        # window 1: attn chunk 1 + scores/exp chunk 2 (+borrowed S(3))
        prefetch_xt(3, eng=nc.sync)
        emit_window(
            [(2, m) for m in range(b0, 12)] + [(3, m) for m in range(b1)],
            at_units(1) + proj_units(3) + vproj_units(2),
        )

        # window 2: attn chunk 2 + scores/exp chunk 3 (tail steps deferred)
        emit_window(
            [(3, m) for m in range(b1, 16 - bl)],
            at_units(2) + vproj_units(3),
        )

        # window 3: attn chunk 3; deferred S(3) steps land just before the
        # attv that consumes them
        late = list(range(16 - bl, 16))
        for i, qt in enumerate((12, 13, 14, 15)):
            for m in late[:]:
                if m <= qt + CFG.get('late_lead', 1) or i == 3:
                    score_step(3, m)
                    late.remove(m)
            for unit in attv_units(3, qt):
                emit(unit)
            norm_unit(qt)
            if 1 <= i <= 2:
                tail_unit(qt - 1, act_assist=(qt - 1 >= 13))
        tail_transp(14, act_assist=True)
        tail_transp(15, act_assist=True)
        tail_po(14, act_assist=True, po_y=True)
        tail_po(15, act_assist=True, po_y=False)

    return nc


_NC_CACHE = None


def kernel(**inputs) -> np.ndarray:
    global _NC_CACHE
    x = np.asarray(inputs["x"], np.float32)
    Wq = np.asarray(inputs["Wq"], np.float32)
    Wk = np.asarray(inputs["Wk"], np.float32)
    Wv = np.asarray(inputs["Wv"], np.float32)
    Wp = np.asarray(inputs["Wp"], np.float32)
    bq = np.asarray(inputs["bq"], np.float32)
    bk = np.asarray(inputs["bk"], np.float32)
    bv = np.asarray(inputs["bv"], np.float32)
    bp = np.asarray(inputs["bp"], np.float32)

    if _NC_CACHE is None:
        _NC_CACHE = build_kernel()
    nc = _NC_CACHE

    def b16(a):
        return np.ascontiguousarray(a).astype(ml_dtypes.bfloat16)

    in_maps = []
    for c in range(NCORES):
        b, g = divmod(c, GROUPS)
        rows = slice(g * DG, (g + 1) * DG)
        in_maps.append({
            "xT": b16(x[b].T),
            "wq": b16(Wq[rows, :].T),
            "wk": b16(Wk[rows, :].T),
            "wv": b16(Wv[rows, :].T),
            "wp": b16(Wp[:, rows].T),
            "bq": np.ascontiguousarray(bq[rows]),
            "bk": np.ascontiguousarray(bk[rows]),
        })

    res = run_bass_kernel_spmd(nc, in_maps, core_ids=list(range(NCORES)))

    result = np.zeros((B, T, C), np.float32)
    for c in range(NCORES):
        b = c // GROUPS
        result[b] += np.asarray(res.results[c]["out"], np.float32)
    result += (bv @ Wp.T + bp)[None, None, :]
    return result
